# revision 30
# baseline (speedup 1.0000x reference)
"""AttentionalPropagation (GNN message passing) Trainium2 Bass kernel.

Reference computation (B=4, D=256, N=M=2048, H=4 heads, head_dim=64):
    q = Wq@x+bq ; k = Wk@source+bk ; v = Wv@source+bv        (conv1x1)
    scores[b,h,n,m] = (q_h . k_h) / 8
    prob = softmax_m(scores) * edge[b,n,m]
    msg  = prob @ v_h   -> merge heads -> Wm@msg+bm
    out  = W2 @ relu(W1 @ [x; message] + b1) + b2

Sharding: 8 cores = (batch b in 0..3) x (query-half in 0..1).
Each core gets x[:, nq-slice], full source, edge[nq-slice, :] (transposed
and cast to fp16 on the host) and computes out[:, nq-slice].

Layout: scores are computed TRANSPOSED ([m, n], m on partitions) so the
exp tiles feed the message matmul directly as the moving operand.  v^T
([m, d], the msg stationary) is produced DIRECTLY by matmuls with the
source tile as the stationary operand (out[m,d] = sum_D src[D,m] Wv^T[D,d])
plus a rank-1 ones x bv matmul for the bias — no PE transposes and no
per-head PSUM->SBUF shuffling.

Engine split per (chunk, head) unit: PE scores+msg+den/bcast; ACT exp
(+ all PSUM->SBUF bias copies: q/k proj, v^T, msg2/out of the MLP); DVE
softmax-denominator adds, 9/16 of the edge multiplies, recip/normalize,
h1 relu; Pool (gpsimd) the other 7 edge multiplies.  The msg matmuls of
unit i are issued after the score matmuls of unit i+1 so PE always has
score work while DVE/Pool finish unit i's multiplies; the chunk-c MLP
is spread one 256-column sub-block per subsequent unit, and the final
chunk's two subs are issued stage-interleaved to pipeline the tail.

PSUM (8 banks): pscore 2x[P,3,512] (score groups AND the v^T waves via
the same ring), pmsg [P,512] (msg rows 0:64, den row 64, bcast 64:128;
also odd q/k projection groups), pmlp [P,512] (MLP half-tiles; even
projection groups).
"""

import os
import numpy as np

import concourse.bass as bass
import concourse.bacc as bacc
import concourse.mybir as mybir
import concourse.tile as tile
from concourse import bass_utils

F32 = mybir.dt.float32
F16 = mybir.dt.float16
AF = mybir.ActivationFunctionType

B, D, N, H = 4, 256, 2048, 4
HD = D // H          # 64
P = 128
NQ = N // 2          # 1024 queries per core
NCORES = 8
NMT = N // P         # 16 m-tiles

# offsets within the packed weight block (fp16 elements)
OFF_WQ, OFF_WK, OFF_WV = 0, 512, 1024
OFF_W1A, OFF_WM, OFF_W1B, OFF_W2 = 1536, 2560, 3072, 4096
WCOLS = 5120
WCOLS_EARLY = 1536   # wq|wk|wv needed for phase 1
XCOLS = 2 * NQ       # 2048
SCOLS = 2 * N        # 4096

LAST_RESULTS = None  # test.py reads this for exec_time_ns

LBL = "?"


def _L(s):
    global LBL
    LBL = s


def build_program(reps: int = 1):
    nc = bacc.Bacc(None, target_bir_lowering=False)

    wpk = nc.dram_tensor("wpk", [P, WCOLS], F16, kind="ExternalInput")
    xpk = nc.dram_tensor("xpk", [P, XCOLS], F16, kind="ExternalInput")
    spk = nc.dram_tensor("spk", [P, SCOLS], F16, kind="ExternalInput")
    edgeT = nc.dram_tensor("edgeT", [N, NQ], F16, kind="ExternalInput")
    bpk = nc.dram_tensor("bpk", [P, 14], F32, kind="ExternalInput")
    bvrow = nc.dram_tensor("bvrow", [1, D], F16, kind="ExternalInput")
    out = nc.dram_tensor("out", [D, NQ], F32, kind="ExternalOutput")

    with tile.TileContext(nc) as tc:
        _loop = tc.For_i(0, reps, 1) if reps > 1 else None
        if _loop is not None:
            _loop.__enter__()
        with (
            tc.tile_pool(name="const", bufs=1) as cp,
            tc.tile_pool(name="w", bufs=1) as wp,
            tc.tile_pool(name="acts", bufs=1) as ap,
            tc.tile_pool(name="pscore", bufs=3, space="PSUM") as pscore,
            tc.tile_pool(name="pmsg", bufs=1, space="PSUM") as pmsg,
            tc.tile_pool(name="pmlp", bufs=1, space="PSUM") as pmlpp,
            tc.tile_pool(name="edgep", bufs=1) as edgep,
            tc.tile_pool(name="up", bufs=3) as up,
            tc.tile_pool(name="accp", bufs=2) as accp,
            tc.tile_pool(name="rdp", bufs=2) as rdp,
            tc.tile_pool(name="outp", bufs=2) as outp,
        ):
            ones16 = cp.tile([P, 1], F16)
            nc.vector.memset(ones16, 1.0)
            ones_row = cp.tile([1, HD], F16)
            nc.vector.memset(ones_row, 1.0)
            ones_rowP = cp.tile([1, P], F16)
            nc.vector.memset(ones_rowP, 1.0)
            bias = cp.tile([P, 14], F32)
            nc.sync.dma_start(out=bias[:, :], in_=bpk[:, :])
            bv_sb = cp.tile([1, D], F16)
            nc.sync.dma_start(out=bv_sb[:, :], in_=bvrow[:, :])

            # [x | weights | src] in one SBUF tile; DMAs split so early
            # consumers start as soon as their slice lands.
            wx_sb = wp.tile([P, WCOLS + XCOLS + SCOLS], F16)
            WOFF = XCOLS
            SOFF = XCOLS + WCOLS
            def dma_cols(dst_off, src, ranges):
                for a, b in ranges:
                    nc.sync.dma_start(out=wx_sb[:, dst_off + a:dst_off + b],
                                      in_=src[:, a:b])

            # ordered so each consumer's kk-pair lands together: wq; x first
            # halves; wk|wv; src first halves; the rest
            dma_cols(WOFF, wpk, [(0, 512)])                      # wq
            dma_cols(0, xpk, [(0, 512), (NQ, NQ + 512)])         # x n0, both kk
            dma_cols(WOFF, wpk, [(512, WCOLS_EARLY)])            # wk|wv
            dma_cols(SOFF, spk, [(0, N // 2), (N, N + N // 2)])  # src n0 kk0/1
            dma_cols(SOFF, spk, [(N // 2, N), (N + N // 2, 2 * N)])
            dma_cols(0, xpk, [(512, NQ), (NQ + 512, 2 * NQ)])    # x n1

            def wview(off, ncols, nk):
                return wx_sb[:, off:off + nk * ncols].rearrange(
                    "p (k c) -> p k c", k=nk)

            x_sb = wview(0, NQ, 2)
            wq_sb = wview(XCOLS + OFF_WQ, D, 2)
            wk_sb = wview(XCOLS + OFF_WK, D, 2)
            wv_sb = wview(XCOLS + OFF_WV, D, 2)
            w1a_sb = wview(XCOLS + OFF_W1A, 2 * D, 2)
            wm_sb = wview(XCOLS + OFF_WM, D, 2)
            w1b_sb = wview(XCOLS + OFF_W1B, 2 * D, 2)
            w2_sb = wview(XCOLS + OFF_W2, D, 4)
            src_sb = wview(SOFF, N, 2)

            q_sb = ap.tile([P, 2, NQ], F16)
            k_sb = ap.tile([P, 2, N], F16)
            # v^T: [m-part, mt, 4h*64d] — stationary tiles for msg matmuls
            vt_sb = ap.tile([P, NMT, D], F16)
            msg_sb = ap.tile([P, 2, NQ], F16)
            msg2_sb = ap.tile([P, 2, NQ], F16)
            h1_sb = ap.tile([P, 4, NQ], F16)

            pmlp = pmlpp.tile([P, 512], F32)
            proj_rot = [0]

            def proj_psum():
                # q/k projection groups alternate between the pmlp and pmsg
                # banks (both free this early) for 2-deep pipelining.
                j = proj_rot[0] % 2
                proj_rot[0] += 1
                if j == 0:
                    return pmlp[:, :]
                ps = pmsg.tile([P, 512], F32, tag="mb", name="projps")
                return ps

            def proj_q(dt_, nchk, dve=False):
                _L(f"projq{dt_}{nchk}")
                ps = proj_psum()
                for kk in range(2):
                    nc.tensor.matmul(
                        ps,
                        wq_sb[:, kk, dt_ * P:(dt_ + 1) * P],
                        x_sb[:, kk, nchk * 512:(nchk + 1) * 512],
                        start=(kk == 0), stop=(kk == 1))
                dst = q_sb[:, dt_, nchk * 512:(nchk + 1) * 512]
                if dve:
                    nc.vector.tensor_scalar_add(dst, ps, bias[:, dt_:dt_ + 1])
                else:
                    nc.scalar.activation(dst, ps, AF.Identity,
                                         bias=bias[:, dt_:dt_ + 1])

            def proj_k(dt_, nchk, dve=False):
                _L(f"projk{dt_}{nchk}")
                ps = proj_psum()
                for kk in range(2):
                    nc.tensor.matmul(
                        ps,
                        wk_sb[:, kk, dt_ * P:(dt_ + 1) * P],
                        src_sb[:, kk, nchk * 512:(nchk + 1) * 512],
                        start=(kk == 0), stop=(kk == 1))
                dst = k_sb[:, dt_, nchk * 512:(nchk + 1) * 512]
                if dve:
                    nc.vector.tensor_scalar_add(dst, ps, bias[:, 2 + dt_:3 + dt_])
                else:
                    nc.scalar.activation(dst, ps, AF.Identity,
                                         bias=bias[:, 2 + dt_:3 + dt_])

            def vt_wave(wave):
                _L(f"vtw{wave}")
                # 4 m-tiles; per m-tile (rank-1 bias, kk0, kk1) accumulate
                # [m, 256] in a quarter of a pscore-ring tile.
                ps = pscore.tile([P, 2, 512], F32, tag="ps2")
                for j in range(4):
                    r = ps[:, j // 2, (j % 2) * 256:(j % 2) * 256 + 256]
                    nc.tensor.matmul(r, ones_rowP[:, :], bv_sb[:, :],
                                     start=True, stop=False)
                for j in range(4):
                    mt = wave * 4 + j
                    r = ps[:, j // 2, (j % 2) * 256:(j % 2) * 256 + 256]
                    for kk in range(2):
                        nc.tensor.matmul(
                            r,
                            src_sb[:, kk, mt * P:(mt + 1) * P],
                            wv_sb[:, kk, :],
                            start=False, stop=(kk == 1))
                nc.vector.tensor_copy(
                    vt_sb[:, wave * 4:(wave + 1) * 4, :].rearrange(
                        "p a c -> p (a c)"),
                    ps[:, 0:2, :].rearrange("p a c -> p (a c)"))

            GROUPS = tuple((2 * i, 2) for i in range(8))
            edge_tiles = []
            for c in range(2):
                edge_t = edgep.tile([P, NMT, 512], F16, tag=f"edge{c}")
                for g in range(4):
                    nc.sync.dma_start(
                        out=edge_t[:, 4 * g:4 * g + 4, :],
                        in_=edgeT[4 * g * P:4 * (g + 1) * P,
                                  c * 512:(c + 1) * 512].rearrange(
                                      "(t p) n -> p t n", p=P))
                edge_tiles.append(edge_t)

            mlp_rot = [0]
            state = {}      # (c, h) -> (u, acc or None)

            def attend(c, h, mode="std", defer=False, hook=None):
                # mode "mid": pool gets only the two leading mul groups so
                # its backlog never delays this unit's msg matmuls.
                # mode "tail": last unit — all muls on DVE, denominator
                # summed on the (otherwise idle) PE into pmlp row 32, no
                # DVE adds at all.
                edge_t = edge_tiles[c]
                hb, ht = HD * (h % 2), h // 2
                qh = q_sb[hb:hb + HD, ht, c * 512:(c + 1) * 512]
                _L(f"attend{c}{h}")
                u = up.tile([P, NMT, 512], F16, tag="u")
                if mode == "std":
                    mul_eng = {g: (nc.gpsimd if g >= 4 else nc.vector)
                               for g in range(8)}
                else:           # mid / tail
                    mul_eng = {g: (nc.gpsimd if g < 3 else nc.vector)
                               for g in range(8)}
                acc = None
                if mode != "tail":
                    acc = accp.tile([P, 2, 512], F16, tag="acc")
                ew_groups = []

                def ew_one(gi):
                    g = 2 * gi
                    if gi == 0:
                        return          # summed together with group 1
                    if gi == 1:
                        nc.vector.tensor_add(acc[:, :, :], u[:, 0:2, :],
                                             u[:, 2:4, :])
                        mul_eng[0].tensor_mul(u[:, 0:2, :], u[:, 0:2, :],
                                              edge_t[:, 0:2, :])
                    else:
                        nc.vector.tensor_add(acc[:, :, :], acc[:, :, :],
                                             u[:, g:g + 2, :])
                    mul_eng[gi].tensor_mul(u[:, g:g + 2, :], u[:, g:g + 2, :],
                                           edge_t[:, g:g + 2, :])
                    if gi == len(GROUPS) - 1:
                        nc.vector.tensor_add(acc[:, 0, :], acc[:, 0, :],
                                             acc[:, 1, :])

                def ew_all():
                    _L(f"attend{c}{h}")
                    for gi in ew_groups:
                        ew_one(gi)
                def den_pe(gi):
                    g0, gn = GROUPS[gi]
                    for j in range(gn):
                        mt = g0 + j
                        nc.tensor.matmul(
                            pmlp[32:33, :], ones16[:, :], u[:, mt, :],
                            start=(mt == 0), stop=(mt == NMT - 1),
                            skip_group_check=True)

                for gi, (g0, gn) in enumerate(GROUPS):
                    ps = pscore.tile([P, 2, 512], F32, tag="ps2")
                    for j in range(gn):
                        mt = g0 + j
                        nc.tensor.matmul(
                            ps[:, j, :],
                            k_sb[hb:hb + HD, ht, mt * P:(mt + 1) * P],
                            qh, start=True, stop=True)
                    nc.scalar.activation(
                        u[:, g0:g0 + gn, :], ps[:, 0:gn, :],
                        AF.Exp, scale=0.125)
                    if gi == 2 and hook is not None:
                        hook()
                        _L(f"attend{c}{h}")
                    if mode == "tail":
                        continue        # den + muls issued below
                    ew_groups.append(gi)
                if mode == "tail":
                    for gi in range(len(GROUPS)):
                        den_pe(gi)
                    for g0 in range(0, NMT, 4):
                        nc.vector.tensor_mul(u[:, g0:g0 + 4, :],
                                             u[:, g0:g0 + 4, :],
                                             edge_t[:, g0:g0 + 4, :])
                state[(c, h)] = (u, acc)
                if defer:
                    return ew_all
                ew_all()
                return None

            fstate = {}

            def finish_a(c, h, nmt_first=8):
                _L(f"finish{c}{h}")
                u, acc = state.pop((c, h))
                # pmsg bank: rows 0:64 msg accum, row 64 den, 64:128 bcast
                mb = pmsg.tile([P, 512], F32, tag="mb")
                if acc is not None:
                    nc.tensor.matmul(mb[HD:HD + 1, :], ones16[:, :],
                                     acc[:, 0, :], start=True, stop=True,
                                     skip_group_check=True)
                    den_row = mb[HD:HD + 1, :]
                else:
                    den_row = pmlp[32:33, :]
                rden = rdp.tile([1, 512], F16, tag="rden")
                with nc.allow_low_precision("fp16 reciprocal of den"):
                    nc.vector.reciprocal(rden[:, :], den_row)
                for mt in range(nmt_first):
                    nc.tensor.matmul(
                        mb[0:HD, :],
                        vt_sb[:, mt, h * HD:(h + 1) * HD],
                        u[:, mt, :],
                        start=(mt == 0), stop=False)
                fstate[(c, h)] = (u, mb, rden, nmt_first)

            def finish_b(c, h):
                _L(f"finish{c}{h}")
                hb, ht = HD * (h % 2), h // 2
                u, mb, rden, nmt_first = fstate.pop((c, h))
                for mt in range(nmt_first, NMT):
                    nc.tensor.matmul(
                        mb[0:HD, :],
                        vt_sb[:, mt, h * HD:(h + 1) * HD],
                        u[:, mt, :],
                        start=False, stop=(mt == NMT - 1))
                nc.tensor.matmul(mb[HD:2 * HD, :], ones_row[:, :],
                                 rden[:, :], start=True, stop=True,
                                 skip_group_check=True)
                rdbc = rdp.tile([HD, 512], F32, tag="rdbc")
                nc.vector.tensor_copy(rdbc[:, :], mb[HD:2 * HD, :])
                nc.vector.tensor_mul(
                    msg_sb[hb:hb + HD, ht, c * 512:(c + 1) * 512],
                    mb[0:HD, :], rdbc[:, :])

            def finish(c, h):
                finish_a(c, h)
                finish_b(c, h)

            deep_psums = []

            def mlp_psum(deep):
                if not deep:
                    j = mlp_rot[0] % 2
                    mlp_rot[0] += 1
                    return pmlp[:, j * 256:(j + 1) * 256]
                # tail: deep-rotate through the freed pscore ring (6
                # quarter-bank regions per tile) so the mm->elementwise
                # chains pipeline without 2-deep WAR stalls
                if not deep_psums:
                    t = pscore.tile([P, 2, 512], F32, tag="ps2")
                    deep_psums.extend(
                        t[:, i // 2, (i % 2) * 256:(i % 2) * 256 + 256]
                        for i in range(4))
                return deep_psums.pop(0)

            def mlp_wm(r, dt_, deep=False):
                _L(f"wm{r.start}.{dt_}")
                ps = mlp_psum(deep)
                for kk in range(2):
                    nc.tensor.matmul(
                        ps,
                        wm_sb[:, kk, dt_ * P:(dt_ + 1) * P],
                        msg_sb[:, kk, r],
                        start=(kk == 0), stop=(kk == 1))
                nc.scalar.activation(
                    msg2_sb[:, dt_, r], ps,
                    AF.Identity, bias=bias[:, 6 + dt_:7 + dt_])

            def mlp_w1(r, dt_, deep=False):
                _L(f"w1.{r.start}.{dt_}")
                ps = mlp_psum(deep)
                for kk in range(2):
                    nc.tensor.matmul(
                        ps,
                        w1a_sb[:, kk, dt_ * P:(dt_ + 1) * P],
                        x_sb[:, kk, r],
                        start=(kk == 0), stop=False)
                for kk in range(2):
                    nc.tensor.matmul(
                        ps,
                        w1b_sb[:, kk, dt_ * P:(dt_ + 1) * P],
                        msg2_sb[:, kk, r],
                        start=False, stop=(kk == 1))
                if dt_ % 2 == 0:
                    nc.scalar.activation(
                        h1_sb[:, dt_, r], ps,
                        AF.Relu, bias=bias[:, 8 + dt_:9 + dt_])
                else:
                    nc.vector.tensor_scalar(
                        h1_sb[:, dt_, r], ps,
                        bias[:, 8 + dt_:9 + dt_], 0.0,
                        op0=mybir.AluOpType.add,
                        op1=mybir.AluOpType.max)

            def mlp_w2(r, dt_, deep=False):
                _L(f"w2.{r.start}.{dt_}")
                ps = mlp_psum(deep)
                for kk in range(4):
                    nc.tensor.matmul(
                        ps,
                        w2_sb[:, kk, dt_ * P:(dt_ + 1) * P],
                        h1_sb[:, kk, r],
                        start=(kk == 0), stop=(kk == 3))
                oc = outp.tile([P, 256], F32)
                nc.scalar.activation(
                    oc[:, :], ps,
                    AF.Identity, bias=bias[:, 12 + dt_:13 + dt_])
                nc.sync.dma_start(
                    out=out[dt_ * P:(dt_ + 1) * P, r],
                    in_=oc[:, :])

            def mlp_tail(c):
                # full-chunk 512-wide MLP through the freed pscore ring:
                # 8 psum regions of [P,512] across 3 ring tiles
                regs = []
                for _ in range(4):
                    t = pscore.tile([P, 2, 512], F32, tag="ps2",
                                    name="tailps")
                    regs.extend(t[:, i, :] for i in range(2))
                r = slice(c * 512, (c + 1) * 512)
                for dt_ in range(2):
                    _L(f"twm{dt_}")
                    ps = regs.pop(0)
                    for kk in range(2):
                        nc.tensor.matmul(
                            ps, wm_sb[:, kk, dt_ * P:(dt_ + 1) * P],
                            msg_sb[:, kk, r],
                            start=(kk == 0), stop=(kk == 1))
                    if dt_ % 2 == 0:
                        nc.scalar.activation(
                            msg2_sb[:, dt_, r], ps,
                            AF.Identity, bias=bias[:, 6 + dt_:7 + dt_])
                    else:
                        nc.vector.tensor_scalar_add(
                            msg2_sb[:, dt_, r], ps, bias[:, 6 + dt_:7 + dt_])
                for dt_ in range(4):
                    _L(f"tw1{dt_}")
                    ps = regs.pop(0)
                    for kk in range(2):
                        nc.tensor.matmul(
                            ps, w1a_sb[:, kk, dt_ * P:(dt_ + 1) * P],
                            x_sb[:, kk, r],
                            start=(kk == 0), stop=False)
                    for kk in range(2):
                        nc.tensor.matmul(
                            ps, w1b_sb[:, kk, dt_ * P:(dt_ + 1) * P],
                            msg2_sb[:, kk, r],
                            start=False, stop=(kk == 1))
                    if dt_ % 2 == 0:
                        nc.scalar.activation(
                            h1_sb[:, dt_, r], ps,
                            AF.Relu, bias=bias[:, 8 + dt_:9 + dt_])
                    else:
                        nc.vector.tensor_scalar(
                            h1_sb[:, dt_, r], ps,
                            bias[:, 8 + dt_:9 + dt_], 0.0,
                            op0=mybir.AluOpType.add,
                            op1=mybir.AluOpType.max)
                for dt_ in range(2):
                    _L(f"tw2{dt_}")
                    ps = regs.pop(0)
                    for kk in range(4):
                        nc.tensor.matmul(
                            ps, w2_sb[:, kk, dt_ * P:(dt_ + 1) * P],
                            h1_sb[:, kk, r],
                            start=(kk == 0), stop=(kk == 3))
                    oc = outp.tile([P, 512], F32, name="octail")
                    if dt_ % 2 == 0:
                        nc.scalar.activation(
                            oc[:, :], ps,
                            AF.Identity, bias=bias[:, 12 + dt_:13 + dt_])
                    else:
                        nc.vector.tensor_scalar_add(
                            oc[:, :], ps, bias[:, 12 + dt_:13 + dt_])
                    nc.sync.dma_start(
                        out=out[dt_ * P:(dt_ + 1) * P, r],
                        in_=oc[:, :])

            def mlp_subs(subs, deep=False):
                # stage-interleaved across the given (c, sub) blocks
                rs = [slice(c * 512 + s * 256, c * 512 + s * 256 + 256)
                      for c, s in subs]
                for dt_ in range(2):
                    for r in rs:
                        mlp_wm(r, dt_, deep)
                for dt_ in range(4):
                    for r in rs:
                        mlp_w1(r, dt_, deep)
                for dt_ in range(2):
                    for r in rs:
                        mlp_w2(r, dt_, deep)

            # ---- issue stream ----
            proj_q(0, 0)
            for nchk in range(4):
                proj_k(0, nchk)
            attend(0, 0)
            proj_q(1, 0, dve=True)
            for nchk in range(4):
                proj_k(1, nchk, dve=True)
            for wave in range(4):
                vt_wave(wave)
            proj_q(0, 1, dve=True)
            proj_q(1, 1, dve=True)
            # late weights (MLP) land while attention spins up
            nc.sync.dma_start(
                out=wx_sb[:, WOFF + WCOLS_EARLY:WOFF + WCOLS],
                in_=wpk[:, WCOLS_EARLY:WCOLS])

            pend = (0, 0)
            mlp_q = []
            for c in range(2):
                for h in range(H):
                    if (c, h) == (0, 0):
                        continue
                    mode = {(1, 2): "mid", (1, 3): "tail"}.get((c, h), "std")
                    if mode == "tail" and pend is not None:
                        finish(*pend)
                        pend = None
                    hook = None
                    if pend is not None:
                        hp = pend

                        def hook():
                            finish_a(*hp)
                    attend(c, h, mode=mode, hook=hook)
                    if pend is not None:
                        finish_b(*pend)
                    if mlp_q:
                        mlp_subs([mlp_q.pop(0)])
                    pend = (c, h)
                    if h == H - 1 and c == 0:
                        mlp_q += [(c, s) for s in range(2)]
            finish(*pend)
            mlp_tail(1)
        if _loop is not None:
            _loop.__exit__(None, None, None)
    nc.finalize()
    return nc


def _pack_rows(a, nk):
    """[nk*128, C] -> [128, nk*C], k-tile-major per partition."""
    c = a.shape[1]
    return np.ascontiguousarray(
        a.reshape(nk, P, c).transpose(1, 0, 2).reshape(P, nk * c))


def prepare_in_maps(inputs):
    x = np.asarray(inputs["x"], np.float32)
    source = np.asarray(inputs["source"], np.float32)
    edge = np.asarray(inputs["edge"], np.float32)
    Wq, bq = np.asarray(inputs["Wq"], np.float32), np.asarray(inputs["bq"], np.float32)
    Wk, bk = np.asarray(inputs["Wk"], np.float32), np.asarray(inputs["bk"], np.float32)
    Wv, bv = np.asarray(inputs["Wv"], np.float32), np.asarray(inputs["bv"], np.float32)
    Wm, bm = np.asarray(inputs["Wm"], np.float32), np.asarray(inputs["bm"], np.float32)
    W1, b1 = np.asarray(inputs["W1"], np.float32), np.asarray(inputs["b1"], np.float32)
    W2, b2 = np.asarray(inputs["W2"], np.float32), np.asarray(inputs["b2"], np.float32)

    # head-major channel permutation: j = h*64+i  <->  c = i*4+h
    perm = np.array([(j % HD) * H + j // HD for j in range(D)])

    f16 = np.float16
    wpk = np.concatenate([
        _pack_rows(Wq[perm].T.astype(f16), 2),
        _pack_rows(Wk[perm].T.astype(f16), 2),
        _pack_rows(Wv[perm].T.astype(f16), 2),
        _pack_rows(W1[:, :D].T.astype(f16), 2),
        _pack_rows(Wm[:, perm].T.astype(f16), 2),
        _pack_rows(W1[:, D:].T.astype(f16), 2),
        _pack_rows(W2.T.astype(f16), 4),
    ], axis=1)
    bpk = np.stack([
        bq[perm][:P], bq[perm][P:], bk[perm][:P], bk[perm][P:],
        bv[perm][:P], bv[perm][P:], bm[:P], bm[P:],
        b1[:P], b1[P:2 * P], b1[2 * P:3 * P], b1[3 * P:],
        b2[:P], b2[P:],
    ], axis=1).astype(np.float32)
    bpk = np.ascontiguousarray(bpk)
    bvrow = np.ascontiguousarray(bv[perm].astype(f16).reshape(1, D))

    shared = {"wpk": wpk, "bpk": bpk, "bvrow": bvrow}
    in_maps = []
    for c in range(NCORES):
        b, half = c // 2, c % 2
        sl = slice(half * NQ, (half + 1) * NQ)
        in_maps.append({
            "xpk": _pack_rows(x[b, :, sl].astype(f16), 2),
            "spk": _pack_rows(source[b].astype(f16), 2),
            "edgeT": np.ascontiguousarray(edge[b, sl, :].T.astype(f16)),
            **shared,
        })
    return in_maps


def kernel(**inputs) -> np.ndarray:
    global LAST_RESULTS
    in_maps = prepare_in_maps(inputs)
    nc = build_program()
    LAST_RESULTS = bass_utils.run_bass_kernel_spmd(
        nc, in_maps, core_ids=list(range(NCORES)),
        trace=os.environ.get("BASS_KERNEL_TRACE", "0") == "1",
    )

    y = np.empty((B, D, N), np.float32)
    for c in range(NCORES):
        b, half = c // 2, c % 2
        y[b, :, half * NQ:(half + 1) * NQ] = LAST_RESULTS.results[c]["out"]
    return y


# revision 40
# speedup vs baseline: 1.7921x; 1.7921x over previous
"""AttentionalPropagation (GNN message passing) Trainium2 Bass kernel.

Reference computation (B=4, D=256, N=M=2048, H=4 heads, head_dim=64):
    q = Wq@x+bq ; k = Wk@source+bk ; v = Wv@source+bv        (conv1x1)
    scores[b,h,n,m] = (q_h . k_h) / 8
    prob = softmax_m(scores) * edge[b,n,m]
    msg  = prob @ v_h   -> merge heads -> Wm@msg+bm
    out  = W2 @ relu(W1 @ [x; message] + b1) + b2

Sharding: 8 cores = (batch b in 0..3) x (query-half in 0..1).
Each core gets x[:, nq-slice], full source, edge[nq-slice, :] (transposed
and cast to fp16 on the host) and computes out[:, nq-slice].

Layout: scores are computed TRANSPOSED ([m, n], m on partitions) so the
exp tiles feed the message matmul directly as the moving operand.  v^T
([m, d], the msg stationary) is produced DIRECTLY by matmuls with the
source tile as the stationary operand (out[m,d] = sum_D src[D,m] Wv^T[D,d])
plus a rank-1 ones x bv matmul for the bias — no PE transposes and no
per-head PSUM->SBUF shuffling.

Engine split per (chunk, head) unit: PE scores+msg+den/bcast; ACT exp
(+ all PSUM->SBUF bias copies: q/k proj, v^T, msg2/out of the MLP); DVE
softmax-denominator adds, 9/16 of the edge multiplies, recip/normalize,
h1 relu; Pool (gpsimd) the other 7 edge multiplies.  The msg matmuls of
unit i are issued after the score matmuls of unit i+1 so PE always has
score work while DVE/Pool finish unit i's multiplies; the chunk-c MLP
is spread one 256-column sub-block per subsequent unit, and the final
chunk's two subs are issued stage-interleaved to pipeline the tail.

PSUM (8 banks): pscore 2x[P,3,512] (score groups AND the v^T waves via
the same ring), pmsg [P,512] (msg rows 0:64, den row 64, bcast 64:128;
also odd q/k projection groups), pmlp [P,512] (MLP half-tiles; even
projection groups).
"""

import os
import numpy as np

import concourse.bass as bass
import concourse.bacc as bacc
import concourse.mybir as mybir
import concourse.tile as tile
from concourse import bass_utils

F32 = mybir.dt.float32
F16 = mybir.dt.float16
AF = mybir.ActivationFunctionType

B, D, N, H = 4, 256, 2048, 4
HD = D // H          # 64
P = 128
NQ = N // 2          # 1024 queries per core
NCORES = 8
NMT = N // P         # 16 m-tiles

# offsets within the packed weight block (fp16 elements)
OFF_WQ, OFF_WK, OFF_WV = 0, 512, 1024
OFF_W1A, OFF_WM, OFF_W1B, OFF_W2 = 1536, 2560, 3072, 4096
WCOLS = 5120
WCOLS_EARLY = 1536   # wq|wk|wv needed for phase 1
XCOLS = 2 * NQ       # 2048
SCOLS = 2 * N        # 4096

LAST_RESULTS = None  # test.py reads this for exec_time_ns

LBL = "?"


def _L(s):
    global LBL
    LBL = s


def build_program(reps: int = 1):
    nc = bacc.Bacc(None, target_bir_lowering=False)

    wpk = nc.dram_tensor("wpk", [P, WCOLS], F16, kind="ExternalInput")
    xpk = nc.dram_tensor("xpk", [P, XCOLS], F16, kind="ExternalInput")
    spk = nc.dram_tensor("spk", [P, SCOLS], F16, kind="ExternalInput")
    edgeT = nc.dram_tensor("edgeT", [N, NQ], F16, kind="ExternalInput")
    bpk = nc.dram_tensor("bpk", [P, 14], F32, kind="ExternalInput")
    bvrow = nc.dram_tensor("bvrow", [1, D], F16, kind="ExternalInput")
    out = nc.dram_tensor("out", [D, NQ], F32, kind="ExternalOutput")

    with tile.TileContext(nc) as tc:
        _loop = tc.For_i(0, reps, 1) if reps > 1 else None
        if _loop is not None:
            _loop.__enter__()
        with (
            tc.tile_pool(name="const", bufs=1) as cp,
            tc.tile_pool(name="w", bufs=1) as wp,
            tc.tile_pool(name="acts", bufs=1) as ap,
            tc.tile_pool(name="pscore", bufs=3, space="PSUM") as pscore,
            tc.tile_pool(name="pmsg", bufs=1, space="PSUM") as pmsg,
            tc.tile_pool(name="pmlp", bufs=1, space="PSUM") as pmlpp,
            tc.tile_pool(name="edgep", bufs=1) as edgep,
            tc.tile_pool(name="up", bufs=5) as up,
            tc.tile_pool(name="accp", bufs=3) as accp,
            tc.tile_pool(name="rdp", bufs=3) as rdp,
            tc.tile_pool(name="outp", bufs=3) as outp,
        ):
            ones16 = cp.tile([P, 1], F16)
            nc.vector.memset(ones16, 1.0)
            ones_row = cp.tile([1, HD], F16)
            nc.vector.memset(ones_row, 1.0)
            ones_rowP = cp.tile([1, P], F16)
            nc.vector.memset(ones_rowP, 1.0)
            bias = cp.tile([P, 14], F32)
            nc.sync.dma_start(out=bias[:, :], in_=bpk[:, :])
            bv_sb = cp.tile([1, D], F16)
            nc.sync.dma_start(out=bv_sb[:, :], in_=bvrow[:, :])

            # [x | weights | src] in one SBUF tile; DMAs split so early
            # consumers start as soon as their slice lands.
            wx_sb = wp.tile([P, WCOLS + XCOLS + SCOLS], F16)
            WOFF = XCOLS
            SOFF = XCOLS + WCOLS
            def dma_cols(dst_off, src, ranges):
                for a, b in ranges:
                    nc.sync.dma_start(out=wx_sb[:, dst_off + a:dst_off + b],
                                      in_=src[:, a:b])

            # ordered so each consumer's kk-pair lands together: wq; x first
            # halves; wk|wv; src first halves; the rest
            dma_cols(WOFF, wpk, [(0, 512)])                      # wq
            dma_cols(0, xpk, [(0, 512), (NQ, NQ + 512)])         # x n0, both kk
            dma_cols(WOFF, wpk, [(512, WCOLS_EARLY)])            # wk|wv
            dma_cols(SOFF, spk, [(0, N // 2), (N, N + N // 2)])  # src n0 kk0/1
            dma_cols(SOFF, spk, [(N // 2, N), (N + N // 2, 2 * N)])
            dma_cols(0, xpk, [(512, NQ), (NQ + 512, 2 * NQ)])    # x n1

            def wview(off, ncols, nk):
                return wx_sb[:, off:off + nk * ncols].rearrange(
                    "p (k c) -> p k c", k=nk)

            x_sb = wview(0, NQ, 2)
            wq_sb = wview(XCOLS + OFF_WQ, D, 2)
            wk_sb = wview(XCOLS + OFF_WK, D, 2)
            wv_sb = wview(XCOLS + OFF_WV, D, 2)
            w1a_sb = wview(XCOLS + OFF_W1A, 2 * D, 2)
            wm_sb = wview(XCOLS + OFF_WM, D, 2)
            w1b_sb = wview(XCOLS + OFF_W1B, 2 * D, 2)
            w2_sb = wview(XCOLS + OFF_W2, D, 4)
            src_sb = wview(SOFF, N, 2)

            q_sb = ap.tile([P, 2, NQ], F16)
            k_sb = ap.tile([P, 2, N], F16)
            # v^T: [m-part, mt, 4h*64d] — stationary tiles for msg matmuls
            vt_sb = ap.tile([P, NMT, D], F16)
            msg_sb = ap.tile([P, 2, NQ], F16)
            msg2_sb = ap.tile([P, 2, NQ], F16)
            h1_sb = ap.tile([P, 4, NQ], F16)

            pmlp = pmlpp.tile([P, 512], F32)
            proj_rot = [0]

            def proj_psum():
                # q/k projection groups alternate between the pmlp and pmsg
                # banks (both free this early) for 2-deep pipelining.
                j = proj_rot[0] % 2
                proj_rot[0] += 1
                if j == 0:
                    return pmlp[:, :]
                ps = pmsg.tile([P, 512], F32, tag="mb", name="projps")
                return ps

            def proj_q(dt_, nchk, dve=False):
                _L(f"projq{dt_}{nchk}")
                ps = proj_psum()
                for kk in range(2):
                    nc.tensor.matmul(
                        ps,
                        wq_sb[:, kk, dt_ * P:(dt_ + 1) * P],
                        x_sb[:, kk, nchk * 512:(nchk + 1) * 512],
                        start=(kk == 0), stop=(kk == 1))
                dst = q_sb[:, dt_, nchk * 512:(nchk + 1) * 512]
                if dve:
                    nc.vector.tensor_scalar_add(dst, ps, bias[:, dt_:dt_ + 1])
                else:
                    nc.scalar.activation(dst, ps, AF.Identity,
                                         bias=bias[:, dt_:dt_ + 1])

            def proj_k(dt_, nchk, dve=False):
                _L(f"projk{dt_}{nchk}")
                ps = proj_psum()
                for kk in range(2):
                    nc.tensor.matmul(
                        ps,
                        wk_sb[:, kk, dt_ * P:(dt_ + 1) * P],
                        src_sb[:, kk, nchk * 512:(nchk + 1) * 512],
                        start=(kk == 0), stop=(kk == 1))
                dst = k_sb[:, dt_, nchk * 512:(nchk + 1) * 512]
                if dve:
                    nc.vector.tensor_scalar_add(dst, ps, bias[:, 2 + dt_:3 + dt_])
                else:
                    nc.scalar.activation(dst, ps, AF.Identity,
                                         bias=bias[:, 2 + dt_:3 + dt_])

            def vt_wave(wave):
                _L(f"vtw{wave}")
                # 4 m-tiles; per m-tile (rank-1 bias, kk0, kk1) accumulate
                # [m, 256] in a quarter of a pscore-ring tile.
                ps = pscore.tile([P, 2, 512], F32, tag="ps2")
                for j in range(4):
                    r = ps[:, j // 2, (j % 2) * 256:(j % 2) * 256 + 256]
                    nc.tensor.matmul(r, ones_rowP[:, :], bv_sb[:, :],
                                     start=True, stop=False)
                for j in range(4):
                    mt = wave * 4 + j
                    r = ps[:, j // 2, (j % 2) * 256:(j % 2) * 256 + 256]
                    for kk in range(2):
                        nc.tensor.matmul(
                            r,
                            src_sb[:, kk, mt * P:(mt + 1) * P],
                            wv_sb[:, kk, :],
                            start=False, stop=(kk == 1))
                nc.vector.tensor_copy(
                    vt_sb[:, wave * 4:(wave + 1) * 4, :].rearrange(
                        "p a c -> p (a c)"),
                    ps[:, 0:2, :].rearrange("p a c -> p (a c)"))

            GROUPS = tuple((2 * i, 2) for i in range(8))
            edge_tiles = []
            for c in range(2):
                edge_t = edgep.tile([P, NMT, 512], F16, tag=f"edge{c}")
                for g in range(4):
                    nc.sync.dma_start(
                        out=edge_t[:, 4 * g:4 * g + 4, :],
                        in_=edgeT[4 * g * P:4 * (g + 1) * P,
                                  c * 512:(c + 1) * 512].rearrange(
                                      "(t p) n -> p t n", p=P))
                edge_tiles.append(edge_t)

            mlp_rot = [0]
            state = {}      # (c, h) -> (u, acc or None)

            def attend(c, h, mode="std", defer=False, hook=None):
                # mode "mid": pool gets only the two leading mul groups so
                # its backlog never delays this unit's msg matmuls.
                # mode "tail": last unit — all muls on DVE, denominator
                # summed on the (otherwise idle) PE into pmlp row 32, no
                # DVE adds at all.
                edge_t = edge_tiles[c]
                hb, ht = HD * (h % 2), h // 2
                qh = q_sb[hb:hb + HD, ht, c * 512:(c + 1) * 512]
                _L(f"attend{c}{h}")
                u = up.tile([P, NMT, 512], F16, tag="u")
                if mode == "std":
                    mul_eng = {g: (nc.gpsimd if g >= 4 else nc.vector)
                               for g in range(8)}
                else:           # mid / tail
                    mul_eng = {g: (nc.gpsimd if g < 3 else nc.vector)
                               for g in range(8)}
                acc = None
                if mode != "tail":
                    acc = accp.tile([P, 2, 512], F16, tag="acc")
                ew_groups = []

                def ew_one(gi):
                    g = 2 * gi
                    if gi == 0:
                        return          # summed together with group 1
                    if gi == 1:
                        nc.vector.tensor_add(acc[:, :, :], u[:, 0:2, :],
                                             u[:, 2:4, :])
                        mul_eng[0].tensor_mul(u[:, 0:2, :], u[:, 0:2, :],
                                              edge_t[:, 0:2, :])
                    else:
                        nc.vector.tensor_add(acc[:, :, :], acc[:, :, :],
                                             u[:, g:g + 2, :])
                    mul_eng[gi].tensor_mul(u[:, g:g + 2, :], u[:, g:g + 2, :],
                                           edge_t[:, g:g + 2, :])
                    if gi == len(GROUPS) - 1:
                        nc.vector.tensor_add(acc[:, 0, :], acc[:, 0, :],
                                             acc[:, 1, :])

                def ew_all():
                    _L(f"attend{c}{h}")
                    for gi in ew_groups:
                        ew_one(gi)
                def den_pe(gi):
                    g0, gn = GROUPS[gi]
                    for j in range(gn):
                        mt = g0 + j
                        nc.tensor.matmul(
                            pmlp[32:33, :], ones16[:, :], u[:, mt, :],
                            start=(mt == 0), stop=(mt == NMT - 1),
                            skip_group_check=True)

                for gi, (g0, gn) in enumerate(GROUPS):
                    ps = pscore.tile([P, 2, 512], F32, tag="ps2")
                    for j in range(gn):
                        mt = g0 + j
                        nc.tensor.matmul(
                            ps[:, j, :],
                            k_sb[hb:hb + HD, ht, mt * P:(mt + 1) * P],
                            qh, start=True, stop=True)
                    nc.scalar.activation(
                        u[:, g0:g0 + gn, :], ps[:, 0:gn, :],
                        AF.Exp, scale=0.125)
                    if gi == 2 and hook is not None:
                        hook()
                        _L(f"attend{c}{h}")
                    if mode == "tail":
                        continue        # den + muls issued below
                    ew_groups.append(gi)
                if mode == "tail":
                    for gi in range(len(GROUPS)):
                        den_pe(gi)
                    for g0 in range(0, NMT, 4):
                        nc.vector.tensor_mul(u[:, g0:g0 + 4, :],
                                             u[:, g0:g0 + 4, :],
                                             edge_t[:, g0:g0 + 4, :])
                state[(c, h)] = (u, acc)
                if defer:
                    return ew_all
                ew_all()
                return None

            fstate = {}

            def finish_a(c, h, nmt_first=8):
                _L(f"finish{c}{h}")
                u, acc = state.pop((c, h))
                # pmsg bank: rows 0:64 msg accum, row 64 den, 64:128 bcast
                mb = pmsg.tile([P, 512], F32, tag="mb")
                if acc is not None:
                    nc.tensor.matmul(mb[HD:HD + 1, :], ones16[:, :],
                                     acc[:, 0, :], start=True, stop=True,
                                     skip_group_check=True)
                    den_row = mb[HD:HD + 1, :]
                else:
                    den_row = pmlp[32:33, :]
                rden = rdp.tile([1, 512], F16, tag="rden")
                with nc.allow_low_precision("fp16 reciprocal of den"):
                    nc.vector.reciprocal(rden[:, :], den_row)
                for mt in range(nmt_first):
                    nc.tensor.matmul(
                        mb[0:HD, :],
                        vt_sb[:, mt, h * HD:(h + 1) * HD],
                        u[:, mt, :],
                        start=(mt == 0), stop=False)
                fstate[(c, h)] = (u, mb, rden, nmt_first)

            def finish_b(c, h):
                _L(f"finish{c}{h}")
                hb, ht = HD * (h % 2), h // 2
                u, mb, rden, nmt_first = fstate.pop((c, h))
                for mt in range(nmt_first, NMT):
                    nc.tensor.matmul(
                        mb[0:HD, :],
                        vt_sb[:, mt, h * HD:(h + 1) * HD],
                        u[:, mt, :],
                        start=False, stop=(mt == NMT - 1))
                nc.tensor.matmul(mb[HD:2 * HD, :], ones_row[:, :],
                                 rden[:, :], start=True, stop=True,
                                 skip_group_check=True)
                rdbc = rdp.tile([HD, 512], F32, tag="rdbc")
                nc.vector.tensor_copy(rdbc[:, :], mb[HD:2 * HD, :])
                nc.vector.tensor_mul(
                    msg_sb[hb:hb + HD, ht, c * 512:(c + 1) * 512],
                    mb[0:HD, :], rdbc[:, :])

            def finish(c, h):
                finish_a(c, h)
                finish_b(c, h)

            deep_psums = []

            def mlp_psum(deep):
                if not deep:
                    j = mlp_rot[0] % 2
                    mlp_rot[0] += 1
                    return pmlp[:, j * 256:(j + 1) * 256]
                # tail: deep-rotate through the freed pscore ring (6
                # quarter-bank regions per tile) so the mm->elementwise
                # chains pipeline without 2-deep WAR stalls
                if not deep_psums:
                    t = pscore.tile([P, 2, 512], F32, tag="ps2")
                    deep_psums.extend(
                        t[:, i // 2, (i % 2) * 256:(i % 2) * 256 + 256]
                        for i in range(4))
                return deep_psums.pop(0)

            def mlp_wm(r, dt_, deep=False):
                _L(f"wm{r.start}.{dt_}")
                ps = mlp_psum(deep)
                for kk in range(2):
                    nc.tensor.matmul(
                        ps,
                        wm_sb[:, kk, dt_ * P:(dt_ + 1) * P],
                        msg_sb[:, kk, r],
                        start=(kk == 0), stop=(kk == 1))
                nc.scalar.activation(
                    msg2_sb[:, dt_, r], ps,
                    AF.Identity, bias=bias[:, 6 + dt_:7 + dt_])

            def mlp_w1(r, dt_, deep=False):
                _L(f"w1.{r.start}.{dt_}")
                ps = mlp_psum(deep)
                for kk in range(2):
                    nc.tensor.matmul(
                        ps,
                        w1a_sb[:, kk, dt_ * P:(dt_ + 1) * P],
                        x_sb[:, kk, r],
                        start=(kk == 0), stop=False)
                for kk in range(2):
                    nc.tensor.matmul(
                        ps,
                        w1b_sb[:, kk, dt_ * P:(dt_ + 1) * P],
                        msg2_sb[:, kk, r],
                        start=False, stop=(kk == 1))
                if dt_ % 2 == 0:
                    nc.scalar.activation(
                        h1_sb[:, dt_, r], ps,
                        AF.Relu, bias=bias[:, 8 + dt_:9 + dt_])
                else:
                    nc.vector.tensor_scalar(
                        h1_sb[:, dt_, r], ps,
                        bias[:, 8 + dt_:9 + dt_], 0.0,
                        op0=mybir.AluOpType.add,
                        op1=mybir.AluOpType.max)

            def mlp_w2(r, dt_, deep=False):
                _L(f"w2.{r.start}.{dt_}")
                ps = mlp_psum(deep)
                for kk in range(4):
                    nc.tensor.matmul(
                        ps,
                        w2_sb[:, kk, dt_ * P:(dt_ + 1) * P],
                        h1_sb[:, kk, r],
                        start=(kk == 0), stop=(kk == 3))
                oc = outp.tile([P, 256], F32)
                nc.scalar.activation(
                    oc[:, :], ps,
                    AF.Identity, bias=bias[:, 12 + dt_:13 + dt_])
                nc.sync.dma_start(
                    out=out[dt_ * P:(dt_ + 1) * P, r],
                    in_=oc[:, :])

            def mlp_tail(c):
                # full-chunk 512-wide MLP through the freed pscore ring:
                # 8 psum regions of [P,512] across 3 ring tiles
                regs = []
                for _ in range(4):
                    t = pscore.tile([P, 2, 512], F32, tag="ps2",
                                    name="tailps")
                    regs.extend(t[:, i, :] for i in range(2))
                r = slice(c * 512, (c + 1) * 512)
                for dt_ in range(2):
                    _L(f"twm{dt_}")
                    ps = regs.pop(0)
                    for kk in range(2):
                        nc.tensor.matmul(
                            ps, wm_sb[:, kk, dt_ * P:(dt_ + 1) * P],
                            msg_sb[:, kk, r],
                            start=(kk == 0), stop=(kk == 1))
                    if dt_ % 2 == 0:
                        nc.scalar.activation(
                            msg2_sb[:, dt_, r], ps,
                            AF.Identity, bias=bias[:, 6 + dt_:7 + dt_])
                    else:
                        nc.vector.tensor_scalar_add(
                            msg2_sb[:, dt_, r], ps, bias[:, 6 + dt_:7 + dt_])
                for dt_ in range(4):
                    _L(f"tw1{dt_}")
                    ps = regs.pop(0)
                    for kk in range(2):
                        nc.tensor.matmul(
                            ps, w1a_sb[:, kk, dt_ * P:(dt_ + 1) * P],
                            x_sb[:, kk, r],
                            start=(kk == 0), stop=False)
                    for kk in range(2):
                        nc.tensor.matmul(
                            ps, w1b_sb[:, kk, dt_ * P:(dt_ + 1) * P],
                            msg2_sb[:, kk, r],
                            start=False, stop=(kk == 1))
                    if dt_ % 2 == 0:
                        nc.scalar.activation(
                            h1_sb[:, dt_, r], ps,
                            AF.Relu, bias=bias[:, 8 + dt_:9 + dt_])
                    else:
                        nc.vector.tensor_scalar(
                            h1_sb[:, dt_, r], ps,
                            bias[:, 8 + dt_:9 + dt_], 0.0,
                            op0=mybir.AluOpType.add,
                            op1=mybir.AluOpType.max)
                for dt_ in range(2):
                    _L(f"tw2{dt_}")
                    ps = regs.pop(0)
                    for kk in range(4):
                        nc.tensor.matmul(
                            ps, w2_sb[:, kk, dt_ * P:(dt_ + 1) * P],
                            h1_sb[:, kk, r],
                            start=(kk == 0), stop=(kk == 3))
                    oc = outp.tile([P, 512], F32, name="octail")
                    if dt_ % 2 == 0:
                        nc.scalar.activation(
                            oc[:, :], ps,
                            AF.Identity, bias=bias[:, 12 + dt_:13 + dt_])
                    else:
                        nc.vector.tensor_scalar_add(
                            oc[:, :], ps, bias[:, 12 + dt_:13 + dt_])
                    nc.sync.dma_start(
                        out=out[dt_ * P:(dt_ + 1) * P, r],
                        in_=oc[:, :])

            def mlp_subs(subs, deep=False):
                # stage-interleaved across the given (c, sub) blocks
                rs = [slice(c * 512 + s * 256, c * 512 + s * 256 + 256)
                      for c, s in subs]
                for dt_ in range(2):
                    for r in rs:
                        mlp_wm(r, dt_, deep)
                for dt_ in range(4):
                    for r in rs:
                        mlp_w1(r, dt_, deep)
                for dt_ in range(2):
                    for r in rs:
                        mlp_w2(r, dt_, deep)

            # ---- issue stream ----
            proj_q(0, 0)
            for nchk in range(4):
                proj_k(0, nchk)
            attend(0, 0)
            proj_q(1, 0, dve=True)
            for nchk in range(4):
                proj_k(1, nchk, dve=True)
            for wave in range(4):
                vt_wave(wave)
            proj_q(0, 1, dve=True)
            proj_q(1, 1, dve=True)
            # late weights (MLP) land while attention spins up
            nc.sync.dma_start(
                out=wx_sb[:, WOFF + WCOLS_EARLY:WOFF + WCOLS],
                in_=wpk[:, WCOLS_EARLY:WCOLS])

            pend = (0, 0)
            mlp_q = []
            for c in range(2):
                for h in range(H):
                    if (c, h) == (0, 0):
                        continue
                    mode = {(1, 2): "mid", (1, 3): "tail"}.get((c, h), "std")
                    if mode == "tail" and pend is not None:
                        finish(*pend)
                        pend = None
                    hook = None
                    if pend is not None:
                        hp = pend

                        def hook():
                            finish_a(*hp)
                    attend(c, h, mode=mode, hook=hook)
                    if pend is not None:
                        finish_b(*pend)
                    if mlp_q:
                        mlp_subs([mlp_q.pop(0)])
                    pend = (c, h)
                    if h == H - 1 and c == 0:
                        mlp_q += [(c, s) for s in range(2)]
            finish(*pend)
            mlp_tail(1)
        if _loop is not None:
            _loop.__exit__(None, None, None)
    nc.finalize()
    return nc


def _pack_rows(a, nk):
    """[nk*128, C] -> [128, nk*C], k-tile-major per partition."""
    c = a.shape[1]
    return np.ascontiguousarray(
        a.reshape(nk, P, c).transpose(1, 0, 2).reshape(P, nk * c))


def prepare_in_maps(inputs):
    x = np.asarray(inputs["x"], np.float32)
    source = np.asarray(inputs["source"], np.float32)
    edge = np.asarray(inputs["edge"], np.float32)
    Wq, bq = np.asarray(inputs["Wq"], np.float32), np.asarray(inputs["bq"], np.float32)
    Wk, bk = np.asarray(inputs["Wk"], np.float32), np.asarray(inputs["bk"], np.float32)
    Wv, bv = np.asarray(inputs["Wv"], np.float32), np.asarray(inputs["bv"], np.float32)
    Wm, bm = np.asarray(inputs["Wm"], np.float32), np.asarray(inputs["bm"], np.float32)
    W1, b1 = np.asarray(inputs["W1"], np.float32), np.asarray(inputs["b1"], np.float32)
    W2, b2 = np.asarray(inputs["W2"], np.float32), np.asarray(inputs["b2"], np.float32)

    # head-major channel permutation: j = h*64+i  <->  c = i*4+h
    perm = np.array([(j % HD) * H + j // HD for j in range(D)])

    f16 = np.float16
    wpk = np.concatenate([
        _pack_rows(Wq[perm].T.astype(f16), 2),
        _pack_rows(Wk[perm].T.astype(f16), 2),
        _pack_rows(Wv[perm].T.astype(f16), 2),
        _pack_rows(W1[:, :D].T.astype(f16), 2),
        _pack_rows(Wm[:, perm].T.astype(f16), 2),
        _pack_rows(W1[:, D:].T.astype(f16), 2),
        _pack_rows(W2.T.astype(f16), 4),
    ], axis=1)
    bpk = np.stack([
        bq[perm][:P], bq[perm][P:], bk[perm][:P], bk[perm][P:],
        bv[perm][:P], bv[perm][P:], bm[:P], bm[P:],
        b1[:P], b1[P:2 * P], b1[2 * P:3 * P], b1[3 * P:],
        b2[:P], b2[P:],
    ], axis=1).astype(np.float32)
    bpk = np.ascontiguousarray(bpk)
    bvrow = np.ascontiguousarray(bv[perm].astype(f16).reshape(1, D))

    shared = {"wpk": wpk, "bpk": bpk, "bvrow": bvrow}
    in_maps = []
    for c in range(NCORES):
        b, half = c // 2, c % 2
        sl = slice(half * NQ, (half + 1) * NQ)
        in_maps.append({
            "xpk": _pack_rows(x[b, :, sl].astype(f16), 2),
            "spk": _pack_rows(source[b].astype(f16), 2),
            "edgeT": np.ascontiguousarray(edge[b, sl, :].T.astype(f16)),
            **shared,
        })
    return in_maps


def kernel(**inputs) -> np.ndarray:
    global LAST_RESULTS
    in_maps = prepare_in_maps(inputs)
    nc = build_program()
    LAST_RESULTS = bass_utils.run_bass_kernel_spmd(
        nc, in_maps, core_ids=list(range(NCORES)),
        trace=os.environ.get("BASS_KERNEL_TRACE", "0") == "1",
    )

    y = np.empty((B, D, N), np.float32)
    for c in range(NCORES):
        b, half = c // 2, c % 2
        y[b, :, half * NQ:(half + 1) * NQ] = LAST_RESULTS.results[c]["out"]
    return y
